# revision 60
# baseline (speedup 1.0000x reference)
"""Trainium2 Bass kernel for nn_AttentionBlock (B=32, F=2048, H=W=7, A=1).

Math (reference):
  xf = x.reshape(B, F, 49)
  q, k, v = split(xf @ W_qkv.T)           # each [B, F, 49]
  S = (q @ k.T) / 7                       # [B, F, F]
  P = softmax(S, axis=-1)
  out = (P @ v) @ W_out.T + b_out         # [B, F, 1]
  out = batchnorm(out, axis=(0, 2)) * gamma + beta

Because A == 1 the output projection commutes into the attention sum:
  w[g]   = v[g] . W_out[0] = xf[g] . u,   u = W_v.T @ W_out[0]   (49-vector)
  out[f] = (sum_g E[f,g] * w[g]) / (sum_g E[f,g]) + b_out,  E = exp(S)
so the device only computes, per (batch, f), the weighted sum and the
denominator.  exp() is computed without max-subtraction (scores are O(1),
|s| < ~15, safely inside fp32 exp range).

Layout: scores are built TRANSPOSED (S_T[g,f] tiles, g on partitions) so
both reductions over g are PE matmuls with a tiny [w_g | 1] stationary:

  per batch:
    xfT [49, 2048]  via PE transposes; duplicated at partitions 64..112
    qT/kT = W{q,k}T.T @ xfT  [49, 2048], duplicated at partitions 64..112
    vw[:, g] = w_g (g<16), vw[:, 16] = 1
    per f-half h (1024 wide), per g-chunk (16 x 128):
      ST[g128, f1024] = kT_g.T @ qT   (2 MMs in row groups 0 / 64 -> run
                                       concurrently in the PE array)
      E = exp(ST / 7)                 (one ScalarE op, reads 2 PSUM banks)
      o[q2] += [w_g | 1].T @ E_half   (accumulating MMs, M=2)

Phase-Q work for batch b+1 (DMA, transposes, projections) is emitted
interleaved into batch b's score loop so the PE never idles waiting on
exp(): idle gaps cause HAM re-throttling to 1.2 GHz (measured 2x).

Batch is data-parallel across the 8 cores (4 batches each); the final
division, bias and the (exact, sync) BatchNorm run on host over the tiny
[32, 2048] result.  Matmuls use float32r (TF32-like single pass).
"""

import numpy as np
from contextlib import ExitStack
from itertools import chain as _chain

import concourse.bass as bass
import concourse.tile as tile
from concourse import bacc, mybir, bass_utils

B, F, HW, A = 32, 2048, 49, 1
N_CORES = 8
BPC = B // N_CORES
SCALE = 1.0 / 7.0
EPS = 1e-5

NF = F // 128               # 16 g-chunks
HALF = 1024
NH = F // HALF              # 2 f-halves
MM_DT = mybir.dt.float32r
PP = True   # run score-pair matmuls concurrently in PE row groups 0 / 64


def _build():
    nc = bacc.Bacc(
        "TRN2",
        target_bir_lowering=False,
        debug=False,
        num_devices=N_CORES,
    )
    f32 = mybir.dt.float32

    xs_d = nc.dram_tensor("xs", [BPC, F, HW], f32, kind="ExternalInput").ap()
    # wqk[:, 0:49] = Wq.T, wqk[:, 64:113] = Wk.T (one merged projection matmul)
    wqk_d = nc.dram_tensor("wqk", [HW, 113], MM_DT, kind="ExternalInput").ap()
    u_d = nc.dram_tensor("u", [HW, 2], MM_DT, kind="ExternalInput").ap()
    ones_d = nc.dram_tensor("ones", [128, 1], MM_DT, kind="ExternalInput").ap()
    id_d = nc.dram_tensor("ident", [128, 128], f32, kind="ExternalInput").ap()
    res_d = nc.dram_tensor("res", [BPC, 2, F], f32, kind="ExternalOutput").ap()

    with tile.TileContext(nc) as tc:
        with ExitStack() as ctx:
            wpool = ctx.enter_context(tc.tile_pool(name="wpool", bufs=1))
            xfp = ctx.enter_context(tc.tile_pool(name="xfp", bufs=2))
            xftp = ctx.enter_context(tc.tile_pool(name="xftp", bufs=2))
            qtp = ctx.enter_context(tc.tile_pool(name="qtp", bufs=2))
            ktp = ctx.enter_context(tc.tile_pool(name="ktp", bufs=2))
            vwp = ctx.enter_context(tc.tile_pool(name="vwp", bufs=2))
            ep = ctx.enter_context(tc.tile_pool(name="ep", bufs=8))
            resp = ctx.enter_context(tc.tile_pool(name="resp", bufs=2))
            pqp = ctx.enter_context(tc.tile_pool(name="pqp", bufs=2, space="PSUM"))
            stp = ctx.enter_context(tc.tile_pool(name="stp", bufs=2, space="PSUM"))
            op = ctx.enter_context(tc.tile_pool(name="op", bufs=1, space="PSUM"))

            wqk_t = wpool.tile([HW, 113], MM_DT)
            u_t = wpool.tile([HW, 2], MM_DT)
            id_t = wpool.tile([128, 128], f32)
            nc.sync.dma_start(out=wqk_t[:], in_=wqk_d)
            nc.sync.dma_start(out=u_t[:], in_=u_d)
            nc.sync.dma_start(out=id_t[:], in_=id_d)

            state = {}

            def phase_q_steps(b):
                """Generator yielding phase-Q work for batch b in small slices.

                Each yielded call emits a few PE ops (plus their DVE copies)
                so the caller can interleave them into another batch's score
                loop, keeping the PE stream dense.
                """
                # xf tiles land on a 64-column pitch (cols 49..63 junk) so a
                # pair of tiles is one contiguous [128, 128] transpose input.
                # 4 chunked DMAs so the first transposes start after 1/4 of
                # the batch's data has landed.
                xf = xfp.tile([128, NF * 64], f32, tag="xf", name=f"xf{b}")
                for dj in range(4):
                    nc.sync.dma_start(
                        out=xf[:, dj * 256 : (dj + 1) * 256]
                        .rearrange("p (t d) -> p t d", d=64)[:, :, 0:HW],
                        in_=xs_d[b, dj * 512 : (dj + 1) * 512].rearrange(
                            "(t p) d -> p t d", p=128
                        ),
                    )
                xfT = xftp.tile([HW, F], MM_DT, tag="xfT", name=f"xfT{b}")
                qT = qtp.tile([128, F], MM_DT, tag="qT", name=f"qT{b}")
                kT = ktp.tile([128, F], MM_DT, tag="kT", name=f"kT{b}")
                vw = vwp.tile([128, NF + 1], MM_DT, tag="vw", name=f"vw{b}")
                nc.sync.dma_start(out=vw[:, NF : NF + 1], in_=ones_d)
                state[b] = (qT, kT, vw)
                yield
                # transposes (two xf tiles per PE op; pair (2t, 2t+1) lands at
                # output partitions 0..48 / 64..112) and the merged q/k
                # projections, interleaved j-wise so the first score matmuls
                # (which need q/k cols 0..1024) unblock after 4 slices.
                def tp_slice(j):
                    tp = pqp.tile([128, 256], f32, tag="pq", name=f"tp{b}_{j}")
                    for tt in range(2):
                        t = 2 * (2 * j + tt)
                        nc.tensor.transpose(
                            tp[:, 128 * tt : 128 * tt + 128],
                            xf[:, t * 64 : (t + 2) * 64],
                            id_t[:],
                        )
                    for tt in range(2):
                        t = 2 * (2 * j + tt)
                        nc.vector.tensor_copy(
                            xfT[:, t * 128 : (t + 1) * 128],
                            tp[0:HW, 128 * tt : 128 * tt + 128],
                        )
                        nc.vector.tensor_copy(
                            xfT[:, (t + 1) * 128 : (t + 2) * 128],
                            tp[64 : 64 + HW, 128 * tt : 128 * tt + 128],
                        )

                def qk_slice(j):
                    sl = slice(j * 512, (j + 1) * 512)
                    qp = pqp.tile([113, 512], f32, tag="pq", name=f"qp{b}_{j}")
                    nc.tensor.matmul(qp[:], wqk_t[:], xfT[:, sl], start=True, stop=True)
                    nc.vector.tensor_copy(qT[0:HW, sl], qp[0:HW, :])
                    nc.vector.tensor_copy(qT[64 : 64 + HW, sl], qp[0:HW, :])
                    nc.vector.tensor_copy(kT[0:HW, sl], qp[64 : 64 + HW, :])
                    nc.vector.tensor_copy(kT[64 : 64 + HW, sl], qp[64 : 64 + HW, :])

                tp_slice(0); yield
                tp_slice(1); yield
                qk_slice(0); yield
                qk_slice(1); yield
                tp_slice(2); yield
                qk_slice(2); yield
                tp_slice(3); yield
                qk_slice(3); yield
                # w row -> column layout without DRAM: park the 4 row-chunks
                # at partitions {0,32,64,96} of w4, then PE-transpose each
                # 128-col block; chunk j lands in transpose-output column 32j.
                w4 = vwp.tile([97, 512], f32, tag="w4", name=f"w4_{b}")
                for j in range(4):
                    sl = slice(j * 512, (j + 1) * 512)
                    wp = pqp.tile([1, 512], f32, tag="pq", name=f"wp{b}_{j}")
                    nc.tensor.matmul(wp[:], u_t[:, 0:1], xfT[:, sl], start=True, stop=True)
                    nc.vector.tensor_copy(w4[32 * j : 32 * j + 1, :], wp[:])
                    yield
                for c in range(4):
                    tp = pqp.tile([128, 97], f32, tag="pq", name=f"wt{b}_{c}")
                    nc.tensor.transpose(
                        tp[:], w4[:, c * 128 : (c + 1) * 128], id_t[0:97, 0:97]
                    )
                    # column 32j of tp holds w for g-chunk 4j + c
                    nc.vector.tensor_copy(vw[:, c : NF : 4], tp[:, 0:97:32])
                    yield

            # Minimal priming for batch 0 (DMA + transposes/projections for
            # q/k cols 0..1024), then one continuous tick pipeline over
            # (batch, half, g): the exp stream starts ~4 slices in instead of
            # after all of phase-Q(0).  Batch 0 h0's O-matmuls run at lag 6
            # so they are emitted after its vw columns exist (program-order
            # dependency rule); everything else runs at lag 2.
            q0 = phase_q_steps(0)
            for _ in range(5):
                next(q0)

            LAG = 2
            LAG0 = 6
            TOT = BPC * NH * NF
            es = {}
            o_ps = {}
            res_tiles = {}
            qnext = q0

            def o_sources(t):
                out = []
                s = t - LAG0
                if 0 <= s < NF:
                    out.append(s)
                s = t - LAG
                if NF <= s < TOT:
                    out.append(s)
                return out

            for t in range(TOT + LAG):
                if t < TOT:
                    b, rem = divmod(t, NH * NF)
                    h, g = divmod(rem, NF)
                    if rem == 0 and b + 1 < BPC:
                        qnext = _chain(qnext, phase_q_steps(b + 1))
                    qT, kT, _ = state[b]
                    st = stp.tile([128, HALF], f32, tag="st", name=f"st{b}_{h}_{g}")
                    for q2 in range(2):
                        base = 64 * q2 if PP else 0
                        nc.tensor.matmul(
                            st[:, q2 * 512 : (q2 + 1) * 512],
                            kT[base : base + HW, g * 128 : (g + 1) * 128],
                            qT[base : base + HW,
                               h * HALF + q2 * 512 : h * HALF + (q2 + 1) * 512],
                            start=True, stop=True, tile_position=(base, 0),
                        )
                for s in o_sources(t):
                    tb, trem = divmod(s, NH * NF)
                    th, tg = divmod(trem, NF)
                    vw = state[tb][2]
                    if tg == 0:
                        o_ps[(tb, th)] = [
                            op.tile([2, 512], f32, tag=f"o{q2}", name=f"o{q2}_{tb}_{th}")
                            for q2 in range(2)
                        ]
                    e_in = es.pop((tb, th, tg))
                    for q2 in range(2):
                        nc.tensor.matmul(
                            o_ps[(tb, th)][q2][:],
                            vw[:, tg : NF + 1 : NF - tg],
                            e_in[:, q2 * 512 : (q2 + 1) * 512],
                            start=(tg == 0), stop=(tg == NF - 1),
                        )
                    if tg == NF - 1:
                        if th == 0:
                            res_tiles[tb] = resp.tile(
                                [2, F], f32, tag="res", name=f"res{tb}"
                            )
                        o_done = o_ps.pop((tb, th))
                        for q2 in range(2):
                            fc = th * 2 + q2
                            nc.vector.tensor_copy(
                                res_tiles[tb][:, fc * 512 : (fc + 1) * 512],
                                o_done[q2][:],
                            )
                        if th == NH - 1:
                            nc.sync.dma_start(out=res_d[tb], in_=res_tiles.pop(tb)[:])
                if t < TOT:
                    e = ep.tile([128, HALF], MM_DT, tag="e", name=f"e{b}_{h}_{g}")
                    es[(b, h, g)] = e
                    nc.scalar.activation(
                        e[:], st[:], mybir.ActivationFunctionType.Exp, scale=SCALE
                    )
                for _ in range(2 if t < 6 else 1):
                    next(qnext, None)

    nc.compile()
    return nc


_NC = None


def _get_nc():
    global _NC
    if _NC is None:
        _NC = _build()
    return _NC


def _run(x, W_qkv, W_out, b_out, gamma, beta, trace=False):
    x = np.asarray(x, dtype=np.float32)
    W_qkv = np.asarray(W_qkv, dtype=np.float32)
    W_out = np.asarray(W_out, dtype=np.float32)
    b_out = np.asarray(b_out, dtype=np.float32)
    gamma = np.asarray(gamma, dtype=np.float32)
    beta = np.asarray(beta, dtype=np.float32)

    xf = np.ascontiguousarray(x.reshape(B, F, HW))
    wqk = np.zeros((HW, 113), dtype=np.float32)
    wqk[:, 0:HW] = W_qkv[0:HW].T
    wqk[:, 64 : 64 + HW] = W_qkv[HW : 2 * HW].T
    u = np.zeros((HW, 2), dtype=np.float32)
    u[:, 0] = W_qkv[2 * HW : 3 * HW].T @ W_out[0]
    ones = np.ones((128, 1), dtype=np.float32)
    ident = np.eye(128, dtype=np.float32)

    in_maps = []
    for c in range(N_CORES):
        in_maps.append(
            {
                "xs": np.ascontiguousarray(xf[c * BPC : (c + 1) * BPC]),
                "wqk": wqk,
                "u": u,
                "ones": ones,
                "ident": ident,
            }
        )

    nc = _get_nc()
    res = bass_utils.run_bass_kernel_spmd(
        nc, in_maps, core_ids=list(range(N_CORES)), trace=trace
    )

    outs = np.empty((B, F), dtype=np.float64)
    for c in range(N_CORES):
        r = res.results[c]["res"]  # [BPC, 2, F]
        num = r[:, 0, :].astype(np.float64)
        den = r[:, 1, :].astype(np.float64)
        outs[c * BPC : (c + 1) * BPC] = num / den
    outs += np.float64(b_out[0])

    # Sync BatchNorm over the batch axis, exactly as the reference.
    mean = outs.mean(axis=0, keepdims=True)
    var = ((outs - mean) ** 2).mean(axis=0, keepdims=True)
    outn = (outs - mean) / np.sqrt(var + EPS)
    outn = outn * gamma[None, :].astype(np.float64) + beta[None, :].astype(np.float64)
    return outn.astype(np.float32).reshape(B, F, A), res


def _run_subprocess(**inputs):
    """Execute _run in a fresh interpreter (fresh PJRT client).

    Used as the retry path: after a transient accelerator fault the
    in-process jax client can be left unusable, but a new process
    re-attaches cleanly.
    """
    import os
    import subprocess
    import sys
    import tempfile

    d = tempfile.mkdtemp(prefix="kretry_")
    inp = os.path.join(d, "in.npz")
    outp = os.path.join(d, "out.npy")
    np.savez(inp, **{k: np.asarray(v) for k, v in inputs.items()})
    code = (
        "import sys, numpy as np\n"
        f"sys.path.insert(0, {os.path.dirname(os.path.abspath(__file__))!r})\n"
        "import kernel as K\n"
        f"z = np.load({inp!r})\n"
        "out, _ = K._run(**{k: z[k] for k in z.files})\n"
        f"np.save({outp!r}, out)\n"
    )
    subprocess.run([sys.executable, "-c", code], check=True, timeout=1500)
    return np.load(outp)


def kernel(x, W_qkv, W_out, b_out, gamma, beta):
    import time

    inputs = dict(x=x, W_qkv=W_qkv, W_out=W_out, b_out=b_out, gamma=gamma, beta=beta)
    try:
        out, _ = _run(**inputs, trace=False)
        return out
    except Exception:
        pass
    last = None
    for attempt in range(2):
        try:
            time.sleep(5.0)
            return _run_subprocess(**inputs)
        except Exception as e:
            last = e
    raise last


# revision 61
# speedup vs baseline: 1.1963x; 1.1963x over previous
"""Trainium2 Bass kernel for nn_AttentionBlock (B=32, F=2048, H=W=7, A=1).

Math (reference):
  xf = x.reshape(B, F, 49)
  q, k, v = split(xf @ W_qkv.T)           # each [B, F, 49]
  S = (q @ k.T) / 7                       # [B, F, F]
  P = softmax(S, axis=-1)
  out = (P @ v) @ W_out.T + b_out         # [B, F, 1]
  out = batchnorm(out, axis=(0, 2)) * gamma + beta

Because A == 1 the output projection commutes into the attention sum:
  w[g]   = v[g] . W_out[0] = xf[g] . u,   u = W_v.T @ W_out[0]   (49-vector)
  out[f] = (sum_g E[f,g] * w[g]) / (sum_g E[f,g]) + b_out,  E = exp(S)
so the device only computes, per (batch, f), the weighted sum and the
denominator.  exp() is computed without max-subtraction (scores are O(1),
|s| < ~15, safely inside fp32 exp range).

Layout: scores are built TRANSPOSED (S_T[g,f] tiles, g on partitions) so
both reductions over g are PE matmuls with a tiny [w_g | 1] stationary:

  per batch:
    xfT [49, 2048]  via PE transposes; duplicated at partitions 64..112
    qT/kT = W{q,k}T.T @ xfT  [49, 2048], duplicated at partitions 64..112
    vw[:, g] = w_g (g<16), vw[:, 16] = 1
    per f-half h (1024 wide), per g-chunk (16 x 128):
      ST[g128, f1024] = kT_g.T @ qT   (2 MMs in row groups 0 / 64 -> run
                                       concurrently in the PE array)
      E = exp(ST / 7)                 (one ScalarE op, reads 2 PSUM banks)
      o[q2] += [w_g | 1].T @ E_half   (accumulating MMs, M=2)

Phase-Q work for batch b+1 (DMA, transposes, projections) is emitted
interleaved into batch b's score loop so the PE never idles waiting on
exp(): idle gaps cause HAM re-throttling to 1.2 GHz (measured 2x).

Batch is data-parallel across the 8 cores (4 batches each); the final
division, bias and the (exact, sync) BatchNorm run on host over the tiny
[32, 2048] result.  Matmuls use float32r (TF32-like single pass).
"""

import numpy as np
from contextlib import ExitStack
from itertools import chain as _chain

import concourse.bass as bass
import concourse.tile as tile
from concourse import bacc, mybir, bass_utils

B, F, HW, A = 32, 2048, 49, 1
N_CORES = 8
BPC = B // N_CORES
SCALE = 1.0 / 7.0
EPS = 1e-5

NF = F // 128               # 16 g-chunks
HALF = 1024
NH = F // HALF              # 2 f-halves
MM_DT = mybir.dt.float32r
PP = True   # run score-pair matmuls concurrently in PE row groups 0 / 64


def _build():
    nc = bacc.Bacc(
        "TRN2",
        target_bir_lowering=False,
        debug=False,
        num_devices=N_CORES,
    )
    f32 = mybir.dt.float32

    xs_d = nc.dram_tensor("xs", [BPC, F, HW], f32, kind="ExternalInput").ap()
    # wqk[:, 0:49] = Wq.T, wqk[:, 64:113] = Wk.T (one merged projection matmul)
    wqk_d = nc.dram_tensor("wqk", [HW, 113], MM_DT, kind="ExternalInput").ap()
    u_d = nc.dram_tensor("u", [HW, 2], MM_DT, kind="ExternalInput").ap()
    ones_d = nc.dram_tensor("ones", [128, 1], MM_DT, kind="ExternalInput").ap()
    id_d = nc.dram_tensor("ident", [128, 128], f32, kind="ExternalInput").ap()
    res_d = nc.dram_tensor("res", [BPC, 2, F], f32, kind="ExternalOutput").ap()

    with tile.TileContext(nc) as tc:
        with ExitStack() as ctx:
            wpool = ctx.enter_context(tc.tile_pool(name="wpool", bufs=1))
            xfp = ctx.enter_context(tc.tile_pool(name="xfp", bufs=2))
            xftp = ctx.enter_context(tc.tile_pool(name="xftp", bufs=2))
            qtp = ctx.enter_context(tc.tile_pool(name="qtp", bufs=2))
            ktp = ctx.enter_context(tc.tile_pool(name="ktp", bufs=2))
            vwp = ctx.enter_context(tc.tile_pool(name="vwp", bufs=2))
            ep = ctx.enter_context(tc.tile_pool(name="ep", bufs=8))
            resp = ctx.enter_context(tc.tile_pool(name="resp", bufs=2))
            pqp = ctx.enter_context(tc.tile_pool(name="pqp", bufs=2, space="PSUM"))
            stp = ctx.enter_context(tc.tile_pool(name="stp", bufs=2, space="PSUM"))
            op = ctx.enter_context(tc.tile_pool(name="op", bufs=1, space="PSUM"))

            wqk_t = wpool.tile([HW, 113], MM_DT)
            u_t = wpool.tile([HW, 2], MM_DT)
            id_t = wpool.tile([128, 128], f32)
            nc.sync.dma_start(out=wqk_t[:], in_=wqk_d)
            nc.sync.dma_start(out=u_t[:], in_=u_d)
            nc.sync.dma_start(out=id_t[:], in_=id_d)

            state = {}

            def phase_q_steps(b):
                """Generator yielding phase-Q work for batch b in small slices.

                Each yielded call emits a few PE ops (plus their DVE copies)
                so the caller can interleave them into another batch's score
                loop, keeping the PE stream dense.
                """
                # xf tiles land on a 64-column pitch (cols 49..63 junk) so a
                # pair of tiles is one contiguous [128, 128] transpose input.
                # 4 chunked DMAs so the first transposes start after 1/4 of
                # the batch's data has landed.
                xf = xfp.tile([128, NF * 64], f32, tag="xf", name=f"xf{b}")
                for dj in range(4):
                    nc.sync.dma_start(
                        out=xf[:, dj * 256 : (dj + 1) * 256]
                        .rearrange("p (t d) -> p t d", d=64)[:, :, 0:HW],
                        in_=xs_d[b, dj * 512 : (dj + 1) * 512].rearrange(
                            "(t p) d -> p t d", p=128
                        ),
                    )
                xfT = xftp.tile([HW, F], MM_DT, tag="xfT", name=f"xfT{b}")
                qT = qtp.tile([128, F], MM_DT, tag="qT", name=f"qT{b}")
                kT = ktp.tile([128, F], MM_DT, tag="kT", name=f"kT{b}")
                vw = vwp.tile([128, NF + 1], MM_DT, tag="vw", name=f"vw{b}")
                nc.sync.dma_start(out=vw[:, NF : NF + 1], in_=ones_d)
                state[b] = (qT, kT, vw)
                yield
                # transposes (two xf tiles per PE op; pair (2t, 2t+1) lands at
                # output partitions 0..48 / 64..112) and the merged q/k
                # projections, interleaved j-wise so the first score matmuls
                # (which need q/k cols 0..1024) unblock after 4 slices.
                def tp_slice(j):
                    tp = pqp.tile([128, 256], f32, tag="pq", name=f"tp{b}_{j}")
                    for tt in range(2):
                        t = 2 * (2 * j + tt)
                        nc.tensor.transpose(
                            tp[:, 128 * tt : 128 * tt + 128],
                            xf[:, t * 64 : (t + 2) * 64],
                            id_t[:],
                        )
                    for tt in range(2):
                        t = 2 * (2 * j + tt)
                        nc.vector.tensor_copy(
                            xfT[:, t * 128 : (t + 1) * 128],
                            tp[0:HW, 128 * tt : 128 * tt + 128],
                        )
                        nc.vector.tensor_copy(
                            xfT[:, (t + 1) * 128 : (t + 2) * 128],
                            tp[64 : 64 + HW, 128 * tt : 128 * tt + 128],
                        )

                def qk_slice(j):
                    sl = slice(j * 512, (j + 1) * 512)
                    qp = pqp.tile([113, 512], f32, tag="pq", name=f"qp{b}_{j}")
                    nc.tensor.matmul(qp[:], wqk_t[:], xfT[:, sl], start=True, stop=True)
                    nc.vector.tensor_copy(qT[0:HW, sl], qp[0:HW, :])
                    nc.vector.tensor_copy(qT[64 : 64 + HW, sl], qp[0:HW, :])
                    nc.vector.tensor_copy(kT[0:HW, sl], qp[64 : 64 + HW, :])
                    nc.vector.tensor_copy(kT[64 : 64 + HW, sl], qp[64 : 64 + HW, :])

                tp_slice(0); yield
                tp_slice(1); yield
                qk_slice(0); yield
                qk_slice(1); yield
                tp_slice(2); yield
                tp_slice(3); yield
                # w row -> column layout without DRAM: park the 4 row-chunks
                # at partitions {0,32,64,96} of w4, then PE-transpose each
                # 128-col block; chunk j lands in transpose-output column 32j.
                w4 = vwp.tile([97, 512], f32, tag="w4", name=f"w4_{b}")
                for j in range(4):
                    sl = slice(j * 512, (j + 1) * 512)
                    wp = pqp.tile([1, 512], f32, tag="pq", name=f"wp{b}_{j}")
                    nc.tensor.matmul(wp[:], u_t[:, 0:1], xfT[:, sl], start=True, stop=True)
                    nc.vector.tensor_copy(w4[32 * j : 32 * j + 1, :], wp[:])
                    yield
                for c in range(4):
                    tp = pqp.tile([128, 97], f32, tag="pq", name=f"wt{b}_{c}")
                    nc.tensor.transpose(
                        tp[:], w4[:, c * 128 : (c + 1) * 128], id_t[0:97, 0:97]
                    )
                    # column 32j of tp holds w for g-chunk 4j + c
                    nc.vector.tensor_copy(vw[:, c : NF : 4], tp[:, 0:97:32])
                    yield
                # q/k cols 1024..2048 last: first needed by the g=8 score
                # matmuls (tick 8), emitted by tick 5
                qk_slice(2); yield
                qk_slice(3); yield

            # Minimal priming for batch 0 (DMA + transposes/projections for
            # q/k cols 0..1024), then one continuous tick pipeline over
            # (batch, half, g): the exp stream starts ~4 slices in instead of
            # after all of phase-Q(0).  Batch 0 h0's O-matmuls run at lag 5
            # so they are emitted after its vw columns exist (program-order
            # dependency rule); everything else runs at lag 2.
            q0 = phase_q_steps(0)
            for _ in range(5):
                next(q0)

            LAG = 2
            LAG0 = 5
            TOT = BPC * NH * NF
            es = {}
            o_ps = {}
            res_tiles = {}
            qnext = q0

            def o_sources(t):
                out = []
                s = t - LAG0
                if 0 <= s < NF:
                    out.append(s)
                s = t - LAG
                if NF <= s < TOT:
                    out.append(s)
                return out

            for t in range(TOT + LAG):
                if t < TOT:
                    b, rem = divmod(t, NH * NF)
                    h, g = divmod(rem, NF)
                    if rem == 0 and b + 1 < BPC:
                        qnext = _chain(qnext, phase_q_steps(b + 1))
                    qT, kT, _ = state[b]
                    st = stp.tile([128, HALF], f32, tag="st", name=f"st{b}_{h}_{g}")
                    for q2 in range(2):
                        base = 64 * q2 if PP else 0
                        nc.tensor.matmul(
                            st[:, q2 * 512 : (q2 + 1) * 512],
                            kT[base : base + HW, g * 128 : (g + 1) * 128],
                            qT[base : base + HW,
                               h * HALF + q2 * 512 : h * HALF + (q2 + 1) * 512],
                            start=True, stop=True, tile_position=(base, 0),
                        )
                for s in o_sources(t):
                    tb, trem = divmod(s, NH * NF)
                    th, tg = divmod(trem, NF)
                    vw = state[tb][2]
                    if tg == 0:
                        o_ps[(tb, th)] = [
                            op.tile([2, 512], f32, tag=f"o{q2}", name=f"o{q2}_{tb}_{th}")
                            for q2 in range(2)
                        ]
                    e_in = es.pop((tb, th, tg))
                    for q2 in range(2):
                        nc.tensor.matmul(
                            o_ps[(tb, th)][q2][:],
                            vw[:, tg : NF + 1 : NF - tg],
                            e_in[:, q2 * 512 : (q2 + 1) * 512],
                            start=(tg == 0), stop=(tg == NF - 1),
                        )
                    if tg == NF - 1:
                        if th == 0:
                            res_tiles[tb] = resp.tile(
                                [2, F], f32, tag="res", name=f"res{tb}"
                            )
                        o_done = o_ps.pop((tb, th))
                        for q2 in range(2):
                            fc = th * 2 + q2
                            nc.vector.tensor_copy(
                                res_tiles[tb][:, fc * 512 : (fc + 1) * 512],
                                o_done[q2][:],
                            )
                        if th == NH - 1:
                            nc.sync.dma_start(out=res_d[tb], in_=res_tiles.pop(tb)[:])
                if t < TOT:
                    e = ep.tile([128, HALF], MM_DT, tag="e", name=f"e{b}_{h}_{g}")
                    es[(b, h, g)] = e
                    nc.scalar.activation(
                        e[:], st[:], mybir.ActivationFunctionType.Exp, scale=SCALE
                    )
                for _ in range(2 if t < 6 else 1):
                    next(qnext, None)

    nc.compile()
    return nc


_NC = None


def _get_nc():
    global _NC
    if _NC is None:
        _NC = _build()
    return _NC


def _run(x, W_qkv, W_out, b_out, gamma, beta, trace=False):
    x = np.asarray(x, dtype=np.float32)
    W_qkv = np.asarray(W_qkv, dtype=np.float32)
    W_out = np.asarray(W_out, dtype=np.float32)
    b_out = np.asarray(b_out, dtype=np.float32)
    gamma = np.asarray(gamma, dtype=np.float32)
    beta = np.asarray(beta, dtype=np.float32)

    xf = np.ascontiguousarray(x.reshape(B, F, HW))
    wqk = np.zeros((HW, 113), dtype=np.float32)
    wqk[:, 0:HW] = W_qkv[0:HW].T
    wqk[:, 64 : 64 + HW] = W_qkv[HW : 2 * HW].T
    u = np.zeros((HW, 2), dtype=np.float32)
    u[:, 0] = W_qkv[2 * HW : 3 * HW].T @ W_out[0]
    ones = np.ones((128, 1), dtype=np.float32)
    ident = np.eye(128, dtype=np.float32)

    in_maps = []
    for c in range(N_CORES):
        in_maps.append(
            {
                "xs": np.ascontiguousarray(xf[c * BPC : (c + 1) * BPC]),
                "wqk": wqk,
                "u": u,
                "ones": ones,
                "ident": ident,
            }
        )

    nc = _get_nc()
    res = bass_utils.run_bass_kernel_spmd(
        nc, in_maps, core_ids=list(range(N_CORES)), trace=trace
    )

    outs = np.empty((B, F), dtype=np.float64)
    for c in range(N_CORES):
        r = res.results[c]["res"]  # [BPC, 2, F]
        num = r[:, 0, :].astype(np.float64)
        den = r[:, 1, :].astype(np.float64)
        outs[c * BPC : (c + 1) * BPC] = num / den
    outs += np.float64(b_out[0])

    # Sync BatchNorm over the batch axis, exactly as the reference.
    mean = outs.mean(axis=0, keepdims=True)
    var = ((outs - mean) ** 2).mean(axis=0, keepdims=True)
    outn = (outs - mean) / np.sqrt(var + EPS)
    outn = outn * gamma[None, :].astype(np.float64) + beta[None, :].astype(np.float64)
    return outn.astype(np.float32).reshape(B, F, A), res


def _run_subprocess(**inputs):
    """Execute _run in a fresh interpreter (fresh PJRT client).

    Used as the retry path: after a transient accelerator fault the
    in-process jax client can be left unusable, but a new process
    re-attaches cleanly.
    """
    import os
    import subprocess
    import sys
    import tempfile

    d = tempfile.mkdtemp(prefix="kretry_")
    inp = os.path.join(d, "in.npz")
    outp = os.path.join(d, "out.npy")
    np.savez(inp, **{k: np.asarray(v) for k, v in inputs.items()})
    code = (
        "import sys, numpy as np\n"
        f"sys.path.insert(0, {os.path.dirname(os.path.abspath(__file__))!r})\n"
        "import kernel as K\n"
        f"z = np.load({inp!r})\n"
        "out, _ = K._run(**{k: z[k] for k in z.files})\n"
        f"np.save({outp!r}, out)\n"
    )
    subprocess.run([sys.executable, "-c", code], check=True, timeout=1500)
    return np.load(outp)


def kernel(x, W_qkv, W_out, b_out, gamma, beta):
    import time

    inputs = dict(x=x, W_qkv=W_qkv, W_out=W_out, b_out=b_out, gamma=gamma, beta=beta)
    try:
        out, _ = _run(**inputs, trace=False)
        return out
    except Exception:
        pass
    last = None
    for attempt in range(2):
        try:
            time.sleep(5.0)
            return _run_subprocess(**inputs)
        except Exception as e:
            last = e
    raise last
